# revision 1
# baseline (speedup 1.0000x reference)
"""Causal Performer (FAVOR+) Trainium2 kernel.

Sharding: 8 cores = 2 (batch) x 4 (head groups of 4 heads).  Each core
computes its batch's projections for its 4 heads, runs the causal
linear-attention scan per head (chunked: intra-block masked matmuls +
cross-block running state), applies its slice of w_o, and returns a
partial [4096, 2048] output.  The host sums the 4 partials per batch.

All matmuls run in bf16 with fp32 PSUM accumulation.  The q-feature
normalization of the reference cancels in numerator/denominator and is
skipped (EPS perturbation is ~1e-5 relative).
"""

import os
import numpy as np
import ml_dtypes

from concourse import bacc, mybir
import concourse.tile as tile
from concourse.bass import ts
from concourse.bass_utils import run_bass_kernel_spmd
from concourse.masks import make_identity

B, S, D = 2, 4096, 2048
H_PER = 4            # heads per core
DK = 128
NB = 8
SBLK = 512           # sequence block
NBLK = S // SBLK     # 8
NSUB = SBLK // 128   # 4 sub-chunks of 128
EPS = 1e-6

bf16 = mybir.dt.bfloat16
f32 = mybir.dt.float32

LAST_EXEC_TIME_NS = None
_CACHE = {}


def _build():
    nc = bacc.Bacc("TRN2", target_bir_lowering=False, debug=False)

    xq_d = nc.dram_tensor("xq", [D, S], bf16, kind="ExternalInput").ap()
    xk_d = nc.dram_tensor("xk", [D, S], bf16, kind="ExternalInput").ap()
    xv_d = nc.dram_tensor("xv", [D, S], bf16, kind="ExternalInput").ap()
    wq_d = nc.dram_tensor("wq", [D, 512], bf16, kind="ExternalInput").ap()
    wk_d = nc.dram_tensor("wk", [D, 512], bf16, kind="ExternalInput").ap()
    wv_d = nc.dram_tensor("wv", [D, 512], bf16, kind="ExternalInput").ap()
    wo_d = nc.dram_tensor("wo", [512, D], bf16, kind="ExternalInput").ap()
    om_d = nc.dram_tensor("om", [DK, NB], bf16, kind="ExternalInput").ap()
    mask_d = nc.dram_tensor("mask", [128, SBLK], f32, kind="ExternalInput").ap()
    part_d = nc.dram_tensor("part", [S, D], f32, kind="ExternalOutput").ap()

    KC = D // 128    # 16 contraction chunks

    with tile.TileContext(nc) as tc:
        with tc.tile_pool(name="const", bufs=1) as const, \
             tc.tile_pool(name="wpool", bufs=1) as wpool, \
             tc.tile_pool(name="state", bufs=1) as state, \
             tc.tile_pool(name="xpool", bufs=1) as xpool, \
             tc.tile_pool(name="qkpool", bufs=2) as qkpool, \
             tc.tile_pool(name="vpool", bufs=2) as vpool, \
             tc.tile_pool(name="atmpool", bufs=6) as atmpool, \
             tc.tile_pool(name="otpool", bufs=6) as otpool, \
             tc.tile_pool(name="osbpool", bufs=2) as osbpool, \
             tc.tile_pool(name="miscpool", bufs=4) as miscpool, \
             tc.tile_pool(name="psbig", bufs=5, space="PSUM") as psbig, \
             tc.tile_pool(name="pssml", bufs=3, space="PSUM") as pssml:

            ident = const.tile([128, 128], bf16, name="ident")
            make_identity(nc, ident)
            mask_sb = const.tile([128, SBLK], f32, name="mask_sb")
            nc.sync.dma_start(mask_sb[:], mask_d[:])
            om_sb = const.tile([DK, NB], bf16, name="om_sb")
            nc.sync.dma_start(om_sb[:], om_d[:])

            wq_sb = wpool.tile([128, KC, 512], bf16, name="wq_sb")
            nc.sync.dma_start(wq_sb[:], wq_d.rearrange("(c p) m -> p c m", p=128))
            wk_sb = wpool.tile([128, KC, 512], bf16, name="wk_sb")
            nc.sync.dma_start(wk_sb[:], wk_d.rearrange("(c p) m -> p c m", p=128))
            wv_sb = wpool.tile([128, KC, 512], bf16, name="wv_sb")
            nc.sync.dma_start(wv_sb[:], wv_d.rearrange("(c p) m -> p c m", p=128))
            wo_sb = wpool.tile([128, H_PER, D], bf16, name="wo_sb")
            nc.sync.dma_start(wo_sb[:], wo_d.rearrange("(c p) m -> p c m", p=128))

            # persistent per-head scan state: cols 0:128 = Z, col 128 = z
            Zsb = []
            Zb16 = []
            for h in range(H_PER):
                zt = state.tile([NB, 132], f32, name=f"Zsb{h}")
                nc.vector.memset(zt[:], 0.0)
                Zsb.append(zt)
                zb = state.tile([NB, 132], bf16, name=f"Zb16_{h}")
                nc.vector.memset(zb[:], 0.0)
                Zb16.append(zb)

            for blk in range(NBLK):
                s0 = blk * SBLK

                xq_sb = xpool.tile([128, KC, SBLK], bf16, name=f"xq{blk}", tag="xq")
                nc.sync.dma_start(
                    xq_sb[:],
                    xq_d.rearrange("(c p) s -> p c s", p=128)[:, :, s0:s0 + SBLK])
                xk_sb = xpool.tile([128, KC, SBLK], bf16, name=f"xk{blk}", tag="xk")
                nc.sync.dma_start(
                    xk_sb[:],
                    xk_d.rearrange("(c p) s -> p c s", p=128)[:, :, s0:s0 + SBLK])
                xv_sb = xpool.tile([128, KC, SBLK], bf16, name=f"xv{blk}", tag="xv")
                nc.sync.dma_start(
                    xv_sb[:],
                    xv_d.rearrange("(c p) s -> p c s", p=128)[:, :, s0:s0 + SBLK])

                # ---- projections ----
                # qhT/khT: [dk(128), head, s(512)]
                qhT = qkpool.tile([128, H_PER, SBLK], bf16, name=f"qhT{blk}", tag="qhT")
                khT = qkpool.tile([128, H_PER, SBLK], bf16, name=f"khT{blk}", tag="khT")
                for dst, wsb, xsb in ((qhT, wq_sb, xq_sb), (khT, wk_sb, xk_sb)):
                    for m in range(H_PER):
                        pp = psbig.tile([128, SBLK], f32, name=f"pj{blk}_{m}", tag="big")
                        for kc in range(KC):
                            nc.tensor.matmul(pp[:], wsb[:, kc, ts(m, 128)],
                                             xsb[:, kc, :],
                                             start=(kc == 0), stop=(kc == KC - 1))
                        nc.scalar.copy(dst[:, m, :], pp[:])

                # vh (+ones col): [s_sub(128), j, head, 132]
                vha = vpool.tile([128, NSUB, H_PER, 132], bf16, name=f"vha{blk}", tag="vha")
                for j in range(NSUB):
                    pp = psbig.tile([128, SBLK], f32, name=f"pv{blk}_{j}", tag="big")
                    for kc in range(KC):
                        nc.tensor.matmul(pp[:], xv_sb[:, kc, ts(j, 128)],
                                         wv_sb[:, kc, :],
                                         start=(kc == 0), stop=(kc == KC - 1))
                    nc.scalar.copy(vha[:, j, :, 0:128],
                                   pp.rearrange("p (h d) -> p h d", d=128))
                    nc.vector.memset(vha[:, j, :, 128:129], 1.0)

                # ---- k features (all heads) ----
                kf_p = pssml.tile([128, 128], f32, name=f"kfp{blk}", tag="sml")
                for h in range(H_PER):
                    for j in range(NSUB):
                        nc.tensor.matmul(kf_p[:, ts(4 * h + j, NB)],
                                         khT[:, h, ts(j, 128)], om_sb[:],
                                         start=True, stop=True)
                kfsq = miscpool.tile([128, 128], f32, name=f"kfsq{blk}", tag="kfsq")
                nc.scalar.square(kfsq[:], kf_p[:])
                kfe = miscpool.tile([128, 128], f32, name=f"kfe{blk}", tag="kfe")
                nc.scalar.activation(kfe[:], kfsq[:],
                                     mybir.ActivationFunctionType.Exp, scale=-0.5)
                kfe3 = kfe.rearrange("p (g n) -> p g n", n=NB)
                ksum = miscpool.tile([128, 16], f32, name=f"ksum{blk}", tag="ksum")
                nc.vector.reduce_sum(ksum[:], kfe3[:], axis=mybir.AxisListType.X)
                nc.vector.tensor_scalar_add(ksum[:], ksum[:], EPS)
                krec = miscpool.tile([128, 16], f32, name=f"krec{blk}", tag="krec")
                nc.vector.reciprocal(krec[:], ksum[:])
                kfn = miscpool.tile([128, 16, NB], bf16, name=f"kfn{blk}", tag="kfn")
                for t in range(16):
                    nc.vector.tensor_scalar(
                        out=kfn[:, t, :], in0=kfe3[:, t, :],
                        scalar1=krec[:, t:t + 1], scalar2=None,
                        op0=mybir.AluOpType.mult)

                # ---- per-head scan ----
                outT = []
                for h in range(H_PER):
                    # q features [nb, s]
                    qfT_p = pssml.tile([NB, SBLK], f32, name=f"qfp{blk}_{h}", tag="sml")
                    nc.tensor.matmul(qfT_p[:], om_sb[:], qhT[:, h, :],
                                     start=True, stop=True)
                    qsq = miscpool.tile([NB, SBLK], f32, name=f"qsq{blk}_{h}", tag="qsq")
                    nc.scalar.square(qsq[:], qfT_p[:])
                    qfT = miscpool.tile([NB, SBLK], bf16, name=f"qfT{blk}_{h}", tag="qfT")
                    nc.scalar.activation(qfT[:], qsq[:],
                                         mybir.ActivationFunctionType.Exp, scale=-0.5)

                    # kfT [nb, s] via PE transpose of kfn columns
                    kfT_p = pssml.tile([NB, SBLK], bf16, name=f"ktp{blk}_{h}", tag="sml")
                    for j in range(NSUB):
                        nc.tensor.transpose(kfT_p[:, ts(j, 128)],
                                            kfn[:, 4 * h + j, :], ident[:])
                    kfT = miscpool.tile([NB, SBLK], bf16, name=f"kfT{blk}_{h}", tag="kfT")
                    nc.vector.tensor_copy(kfT[:], kfT_p[:])

                    # AT blocks [t, s], causally masked
                    atm = []
                    for i2 in range(NSUB):
                        n_i = SBLK - 128 * i2
                        at_p = psbig.tile([128, n_i], f32, name=f"at{blk}_{h}_{i2}", tag="big")
                        nc.tensor.matmul(at_p[:], kfT[:, ts(i2, 128)],
                                         qfT[:, 128 * i2:SBLK], start=True, stop=True)
                        am = atmpool.tile([128, SBLK], bf16, name=f"am{blk}_{h}_{i2}", tag="atm")
                        nc.vector.tensor_mul(am[:, :n_i], at_p[:], mask_sb[:, :n_i])
                        atm.append(am)

                    # num+den per s-sub, then divide, then transpose
                    oT_p = psbig.tile([128, SBLK], bf16, name=f"otp{blk}_{h}", tag="big")
                    for j in range(NSUB):
                        nd = psbig.tile([128, 132], f32, name=f"nd{blk}_{h}_{j}", tag="big")
                        for i2 in range(j + 1):
                            nc.tensor.matmul(nd[:, 0:129],
                                             atm[i2][:, 128 * (j - i2):128 * (j - i2) + 128],
                                             vha[:, i2, h, 0:129],
                                             start=(i2 == 0), stop=False)
                        nc.tensor.matmul(nd[:, 0:129], qfT[:, ts(j, 128)],
                                         Zb16[h][:, 0:129], start=False, stop=True)
                        den = miscpool.tile([128, 1], f32, name=f"den{blk}_{h}_{j}", tag="den")
                        nc.vector.tensor_scalar_add(den[:], nd[:, 128:129], EPS)
                        rec = miscpool.tile([128, 1], f32, name=f"rec{blk}_{h}_{j}", tag="rec")
                        nc.vector.reciprocal(rec[:], den[:])
                        oh = miscpool.tile([128, 128], bf16, name=f"oh{blk}_{h}_{j}", tag="oh")
                        nc.vector.tensor_scalar(
                            out=oh[:], in0=nd[:, 0:128], scalar1=rec[:],
                            scalar2=None, op0=mybir.AluOpType.mult)
                        nc.tensor.transpose(oT_p[:, ts(j, 128)], oh[:], ident[:])

                    oT = otpool.tile([128, SBLK], bf16, name=f"oT{blk}_{h}", tag="outT")
                    nc.vector.tensor_copy(oT[:], oT_p[:])
                    outT.append(oT)

                    # state update (after the inter reads of this block)
                    su = pssml.tile([NB, 132], f32, name=f"su{blk}_{h}", tag="sml")
                    for i2 in range(NSUB):
                        nc.tensor.matmul(su[:, 0:129], kfn[:, 4 * h + i2, :],
                                         vha[:, i2, h, 0:129],
                                         start=(i2 == 0), stop=(i2 == NSUB - 1))
                    nc.vector.tensor_add(Zsb[h][:, 0:129], Zsb[h][:, 0:129], su[:, 0:129])
                    nc.vector.tensor_copy(Zb16[h][:, 0:129], Zsb[h][:, 0:129])

                # ---- output projection ----
                for j in range(NSUB):
                    osb = osbpool.tile([128, D], f32, name=f"osb{blk}_{j}", tag="osb")
                    for c in range(4):
                        op = psbig.tile([128, 512], f32, name=f"op{blk}_{j}_{c}", tag="big")
                        for h in range(H_PER):
                            nc.tensor.matmul(op[:], outT[h][:, ts(j, 128)],
                                             wo_sb[:, h, ts(c, 512)],
                                             start=(h == 0), stop=(h == H_PER - 1))
                        nc.scalar.copy(osb[:, ts(c, 512)], op[:])
                    r0 = s0 + 128 * j
                    nc.sync.dma_start(part_d[r0:r0 + 128, :], osb[:])

    nc.compile()
    return nc


def _prep_inputs(q, k, v, w_q, w_k, w_v, w_o, omega):
    """Host-side sharding: returns in_maps for the 8 cores."""
    bf = ml_dtypes.bfloat16
    mask = np.ones((128, SBLK), np.float32)
    mask[:, :128] = np.triu(np.ones((128, 128), np.float32))
    om_t = np.ascontiguousarray(omega.T).astype(bf)          # [128, 8]

    xs = []
    for b in range(B):
        xs.append((np.ascontiguousarray(q[b].T).astype(bf),
                   np.ascontiguousarray(k[b].T).astype(bf),
                   np.ascontiguousarray(v[b].T).astype(bf)))

    in_maps = []
    for core in range(8):
        b, g = divmod(core, 4)
        sl = slice(512 * g, 512 * (g + 1))
        xq, xk, xv = xs[b]
        in_maps.append({
            "xq": xq, "xk": xk, "xv": xv,
            "wq": np.ascontiguousarray(w_q[sl, :].T).astype(bf),
            "wk": np.ascontiguousarray(w_k[sl, :].T).astype(bf),
            "wv": np.ascontiguousarray(w_v[sl, :].T).astype(bf),
            "wo": np.ascontiguousarray(w_o[:, sl].T).astype(bf),
            "om": om_t,
            "mask": mask,
        })
    return in_maps


def kernel(q, k, v, w_q, w_k, w_v, w_o, omega):
    global LAST_EXEC_TIME_NS
    q, k, v = np.asarray(q), np.asarray(k), np.asarray(v)
    w_q, w_k, w_v, w_o = (np.asarray(a) for a in (w_q, w_k, w_v, w_o))
    omega = np.asarray(omega)

    if "nc" not in _CACHE:
        _CACHE["nc"] = _build()
    nc = _CACHE["nc"]

    in_maps = _prep_inputs(q, k, v, w_q, w_k, w_v, w_o, omega)
    trace = bool(os.environ.get("BASS_KERNEL_TRACE"))
    res = run_bass_kernel_spmd(nc, in_maps, core_ids=list(range(8)), trace=trace)
    LAST_EXEC_TIME_NS = res.exec_time_ns

    out = np.zeros((B, S, D), np.float32)
    for core in range(8):
        b = core // 4
        out[b] += res.results[core]["part"]
    return out


# revision 4
# speedup vs baseline: 1.0609x; 1.0609x over previous
"""Causal Performer (FAVOR+) Trainium2 kernel.

Sharding: 8 cores = 2 (batch) x 4 (head groups of 4 heads).  Each core
computes its 4 heads for one batch and returns a partial [4096, 2048]
output (its heads' contribution through w_o); the host sums the 4
partials per batch.

Key algebraic moves:
  - q/k head projections are FUSED with the random-feature map on the
    host: qf = (q @ Wq_h.T) @ omega.T == q @ (omega @ Wq_h).T, so the
    on-chip contraction produces only 8 features per head (32 per core)
    instead of 128 head dims -- 16x less PE work than materializing qh.
  - The reference's q-feature normalization cancels in num/den and is
    skipped; k-feature normalization is folded into the masked-A^T copy
    as a per-partition scale (scalar_tensor_tensor).
  - Causal scan is chunked: within a 512-block, masked A^T matmuls
    handle intra-block pairs; an [8, 129] running state (Z | z) carries
    history across blocks (updated via matmul, accumulated in fp32).

All matmuls bf16 with fp32 PSUM accumulation.
"""

import os
import numpy as np
import ml_dtypes

from concourse import bacc, mybir
import concourse.tile as tile
from concourse.bass import ts
from concourse.bass_utils import run_bass_kernel_spmd
from concourse.masks import make_identity

B, S, D = 2, 4096, 2048
H_PER = 4            # heads per core
DK = 128
NB = 8
NF = NB * H_PER      # 32 fused feature dims per core
SBLK = 512           # sequence block
NBLK = S // SBLK     # 8
NSUB = SBLK // 128   # 4 sub-chunks of 128
EPS = 1e-6

bf16 = mybir.dt.bfloat16
f32 = mybir.dt.float32

LAST_EXEC_TIME_NS = None
_CACHE = {}


def _build():
    nc = bacc.Bacc("TRN2", target_bir_lowering=False, debug=False)

    xq_d = nc.dram_tensor("xq", [D, S], bf16, kind="ExternalInput").ap()
    xk_d = nc.dram_tensor("xk", [D, S], bf16, kind="ExternalInput").ap()
    xv_d = nc.dram_tensor("xv", [D, S], bf16, kind="ExternalInput").ap()
    wqom_d = nc.dram_tensor("wqom", [D, 128], bf16, kind="ExternalInput").ap()
    wkom_d = nc.dram_tensor("wkom", [D, 128], bf16, kind="ExternalInput").ap()
    wv_d = nc.dram_tensor("wv", [D, 512], bf16, kind="ExternalInput").ap()
    wo_d = nc.dram_tensor("wo", [512, D], bf16, kind="ExternalInput").ap()
    mask_d = nc.dram_tensor("mask", [128, SBLK], f32, kind="ExternalInput").ap()
    part_d = nc.dram_tensor("part", [S, D], f32, kind="ExternalOutput").ap()

    KC = D // 128    # 16 contraction chunks

    with tile.TileContext(nc) as tc:
        with tc.tile_pool(name="const", bufs=1) as const, \
             tc.tile_pool(name="wpool", bufs=1) as wpool, \
             tc.tile_pool(name="state", bufs=1) as state, \
             tc.tile_pool(name="xpool", bufs=1) as xpool, \
             tc.tile_pool(name="vpool", bufs=2) as vpool, \
             tc.tile_pool(name="featpool", bufs=6) as featpool, \
             tc.tile_pool(name="atmpool", bufs=6) as atmpool, \
             tc.tile_pool(name="otpool", bufs=6) as otpool, \
             tc.tile_pool(name="osbpool", bufs=2) as osbpool, \
             tc.tile_pool(name="miscpool", bufs=6) as miscpool, \
             tc.tile_pool(name="psbig", bufs=5, space="PSUM") as psbig, \
             tc.tile_pool(name="pssml", bufs=3, space="PSUM") as pssml:

            ident = const.tile([128, 128], bf16, name="ident")
            make_identity(nc, ident)
            mask_sb = const.tile([128, SBLK], f32, name="mask_sb")
            nc.sync.dma_start(mask_sb[:], mask_d[:])
            ones_col = const.tile([128, 1], bf16, name="ones_col")
            nc.vector.memset(ones_col[:], 1.0)
            ones_row = const.tile([1, 128], bf16, name="ones_row")
            nc.vector.memset(ones_row[:], 1.0)

            wqom_sb = wpool.tile([128, KC, 128], bf16, name="wqom_sb")
            nc.sync.dma_start(wqom_sb[:], wqom_d.rearrange("(c p) m -> p c m", p=128))
            wkom_sb = wpool.tile([128, KC, 128], bf16, name="wkom_sb")
            nc.sync.dma_start(wkom_sb[:], wkom_d.rearrange("(c p) m -> p c m", p=128))
            wv_sb = wpool.tile([128, KC, 512], bf16, name="wv_sb")
            nc.sync.dma_start(wv_sb[:], wv_d.rearrange("(c p) m -> p c m", p=128))
            wo_sb = wpool.tile([128, H_PER, D], bf16, name="wo_sb")
            nc.sync.dma_start(wo_sb[:], wo_d.rearrange("(c p) m -> p c m", p=128))

            # persistent per-head scan state: cols 0:128 = Z, col 128 = z
            Zsb = []
            Zb16 = []
            for h in range(H_PER):
                zt = state.tile([NB, 132], f32, name=f"Zsb{h}")
                nc.vector.memset(zt[:], 0.0)
                Zsb.append(zt)
                zb = state.tile([NB, 132], bf16, name=f"Zb16_{h}")
                nc.vector.memset(zb[:], 0.0)
                Zb16.append(zb)

            for blk in range(NBLK):
                s0 = blk * SBLK

                xq_sb = xpool.tile([128, KC, SBLK], bf16, name=f"xq{blk}", tag="xq")
                nc.sync.dma_start(
                    xq_sb[:],
                    xq_d.rearrange("(c p) s -> p c s", p=128)[:, :, s0:s0 + SBLK])
                xk_sb = xpool.tile([128, KC, SBLK], bf16, name=f"xk{blk}", tag="xk")
                nc.sync.dma_start(
                    xk_sb[:],
                    xk_d.rearrange("(c p) s -> p c s", p=128)[:, :, s0:s0 + SBLK])
                xv_sb = xpool.tile([128, KC, SBLK], bf16, name=f"xv{blk}", tag="xv")
                nc.sync.dma_start(
                    xv_sb[:],
                    xv_d.rearrange("(c p) s -> p c s", p=128)[:, :, s0:s0 + SBLK])

                # ---- v projection: vha [s_sub(128), j, head, 132] (+ones col) ----
                vha = vpool.tile([128, NSUB, H_PER, 132], bf16, name=f"vha{blk}", tag="vha")
                for j in range(NSUB):
                    pp = psbig.tile([128, SBLK], f32, name=f"pv{blk}_{j}", tag="big")
                    for kc in range(KC):
                        nc.tensor.matmul(pp[:], xv_sb[:, kc, ts(j, 128)],
                                         wv_sb[:, kc, :],
                                         start=(kc == 0), stop=(kc == KC - 1))
                    nc.scalar.copy(vha[:, j, :, 0:128],
                                   pp.rearrange("p (h d) -> p h d", d=128))
                    nc.vector.memset(vha[:, j, :, 128:129], 1.0)

                # ---- fused q/k feature projections: [32, 512] ----
                qf_p = pssml.tile([128, SBLK], f32, name=f"qfp{blk}", tag="sml")
                kf_p = pssml.tile([128, SBLK], f32, name=f"kfp{blk}", tag="sml")
                for dst, wsb, xsb in ((qf_p, wqom_sb, xq_sb), (kf_p, wkom_sb, xk_sb)):
                    for kc in range(KC):
                        nc.tensor.matmul(dst[:], wsb[:, kc, :], xsb[:, kc, :],
                                         start=(kc == 0), stop=(kc == KC - 1))
                qsq = miscpool.tile([128, SBLK], f32, name=f"qsq{blk}", tag="qsq")
                nc.scalar.square(qsq[:], qf_p[:])
                ksq = miscpool.tile([128, SBLK], f32, name=f"ksq{blk}", tag="ksq")
                nc.scalar.square(ksq[:], kf_p[:])

                # per-head exp slices -> base-partition-0 tiles [8, 512]
                qfT = []
                kfTu = []
                for h in range(H_PER):
                    qt = featpool.tile([NB, SBLK], bf16, name=f"qfT{blk}_{h}", tag="qfT")
                    nc.scalar.activation(qt[:], qsq[32 * h:32 * h + NB, :],
                                         mybir.ActivationFunctionType.Exp, scale=-0.5)
                    qfT.append(qt)
                    kt = featpool.tile([NB, SBLK], bf16, name=f"kfT{blk}_{h}", tag="kfT")
                    nc.scalar.activation(kt[:], ksq[32 * h:32 * h + NB, :],
                                         mybir.ActivationFunctionType.Exp, scale=-0.5)
                    kfTu.append(kt)

                # ---- per-head scan ----
                outT = []
                for h in range(H_PER):
                    # kf [s, nb] via PE transpose of kfTu; then normalizer
                    kfp2 = pssml.tile([128, NSUB, NB], bf16, name=f"kfp2{blk}_{h}", tag="sml")
                    for j in range(NSUB):
                        nc.tensor.transpose(kfp2[:, j, :], kfTu[h][:, ts(j, 128)],
                                            ident[0:NB, 0:NB])
                    kfu = featpool.tile([128, NSUB, NB], bf16, name=f"kfu{blk}_{h}", tag="kfu")
                    nc.vector.tensor_copy(kfu[:], kfp2[:])
                    ksum = miscpool.tile([128, NSUB], f32, name=f"ksum{blk}_{h}", tag="ksum")
                    nc.vector.reduce_sum(ksum[:], kfu[:], axis=mybir.AxisListType.X)
                    nc.vector.tensor_scalar_add(ksum[:], ksum[:], EPS)
                    krec = miscpool.tile([128, NSUB], f32, name=f"krec{blk}_{h}", tag="krec")
                    nc.vector.reciprocal(krec[:], ksum[:])
                    kfn = featpool.tile([128, NSUB, NB], bf16, name=f"kfn{blk}_{h}", tag="kfn")
                    for j in range(NSUB):
                        nc.vector.tensor_scalar(
                            out=kfn[:, j, :], in0=kfu[:, j, :],
                            scalar1=krec[:, j:j + 1], scalar2=None,
                            op0=mybir.AluOpType.mult)

                    # masked A^T blocks, normalization folded in as row scale
                    atm = []
                    for i2 in range(NSUB):
                        n_i = SBLK - 128 * i2
                        at_p = psbig.tile([128, n_i], f32, name=f"at{blk}_{h}_{i2}", tag="big")
                        nc.tensor.matmul(at_p[:], kfTu[h][:, ts(i2, 128)],
                                         qfT[h][:, 128 * i2:SBLK], start=True, stop=True)
                        am = atmpool.tile([128, SBLK], bf16, name=f"am{blk}_{h}_{i2}", tag="atm")
                        nc.vector.scalar_tensor_tensor(
                            out=am[:, :n_i], in0=at_p[:], scalar=krec[:, i2:i2 + 1],
                            in1=mask_sb[:, :n_i],
                            op0=mybir.AluOpType.mult, op1=mybir.AluOpType.mult)
                        atm.append(am)

                    # numerator^T [d, s]
                    numT = psbig.tile([128, SBLK], f32, name=f"numT{blk}_{h}", tag="big")
                    for i2 in range(NSUB):
                        nc.tensor.matmul(numT[:, 128 * i2:SBLK],
                                         vha[:, i2, h, 0:128], atm[i2][:, :SBLK - 128 * i2],
                                         start=(i2 == 0), stop=False)
                    nc.tensor.matmul(numT[:], Zb16[h][:, 0:128], qfT[h][:],
                                     start=False, stop=True)

                    # denominator [1, s]
                    den_p = pssml.tile([1, SBLK], f32, name=f"den{blk}_{h}", tag="sml")
                    for i2 in range(NSUB):
                        nc.tensor.matmul(den_p[:, 128 * i2:SBLK],
                                         ones_col[:], atm[i2][:, :SBLK - 128 * i2],
                                         start=(i2 == 0), stop=False)
                    nc.tensor.matmul(den_p[:], Zb16[h][:, 128:129], qfT[h][:],
                                     start=False, stop=True)
                    dre = miscpool.tile([1, SBLK], f32, name=f"dre{blk}_{h}", tag="dre")
                    nc.vector.tensor_scalar_add(dre[:], den_p[:], EPS)
                    drr = miscpool.tile([1, SBLK], f32, name=f"drr{blk}_{h}", tag="drr")
                    nc.vector.reciprocal(drr[:], dre[:])
                    drb = miscpool.tile([1, SBLK], bf16, name=f"drb{blk}_{h}", tag="drb")
                    nc.vector.tensor_copy(drb[:], drr[:])

                    # broadcast recip across partitions and divide
                    bc_p = psbig.tile([128, SBLK], f32, name=f"bcp{blk}_{h}", tag="big")
                    nc.tensor.matmul(bc_p[:], ones_row[:], drb[:], start=True, stop=True)
                    bc_sb = miscpool.tile([128, SBLK], bf16, name=f"bcs{blk}_{h}", tag="bcs")
                    nc.vector.tensor_copy(bc_sb[:], bc_p[:])
                    oT = otpool.tile([128, SBLK], bf16, name=f"oT{blk}_{h}", tag="outT")
                    nc.vector.scalar_tensor_tensor(
                        out=oT[:], in0=numT[:], scalar=1.0, in1=bc_sb[:],
                        op0=mybir.AluOpType.mult, op1=mybir.AluOpType.mult)
                    outT.append(oT)

                    # state update (reads of Zb16 above precede these writes)
                    su = pssml.tile([NB, 132], f32, name=f"su{blk}_{h}", tag="sml")
                    for i2 in range(NSUB):
                        nc.tensor.matmul(su[:, 0:129], kfn[:, i2, :],
                                         vha[:, i2, h, 0:129],
                                         start=(i2 == 0), stop=(i2 == NSUB - 1))
                    nc.vector.tensor_add(Zsb[h][:, 0:129], Zsb[h][:, 0:129], su[:, 0:129])
                    nc.vector.tensor_copy(Zb16[h][:, 0:129], Zsb[h][:, 0:129])

                # ---- output projection ----
                for j in range(NSUB):
                    osb = osbpool.tile([128, D], f32, name=f"osb{blk}_{j}", tag="osb")
                    for c in range(4):
                        op = psbig.tile([128, 512], f32, name=f"op{blk}_{j}_{c}", tag="big")
                        for h in range(H_PER):
                            nc.tensor.matmul(op[:], outT[h][:, ts(j, 128)],
                                             wo_sb[:, h, ts(c, 512)],
                                             start=(h == 0), stop=(h == H_PER - 1))
                        nc.scalar.copy(osb[:, ts(c, 512)], op[:])
                    r0 = s0 + 128 * j
                    nc.sync.dma_start(part_d[r0:r0 + 128, :], osb[:])

    nc.compile()
    return nc


def _pad_feat(w):
    """[4, 8, D] head-feature weights -> [D, 128] with head h at cols 32h."""
    out = np.zeros((128, D), np.float32)
    for h in range(H_PER):
        out[32 * h:32 * h + NB] = w[h]
    return np.ascontiguousarray(out.T)


def _prep_inputs(q, k, v, w_q, w_k, w_v, w_o, omega):
    """Host-side sharding: returns in_maps for the 8 cores."""
    bf = ml_dtypes.bfloat16
    mask = np.ones((128, SBLK), np.float32)
    mask[:, :128] = np.triu(np.ones((128, 128), np.float32))

    xs = []
    for b in range(B):
        xs.append((np.ascontiguousarray(q[b].T).astype(bf),
                   np.ascontiguousarray(k[b].T).astype(bf),
                   np.ascontiguousarray(v[b].T).astype(bf)))

    # fused feature projections: per head, omega @ Wq_head  -> [8, 2048]
    # stacked per group -> [32, 2048], transposed -> [2048, 32]
    wq_h = w_q.reshape(16, DK, D)                 # [head, dk, d_in]
    wk_h = w_k.reshape(16, DK, D)
    wqom = np.einsum('nd,hde->hne', omega, wq_h)  # [16, 8, D]
    wkom = np.einsum('nd,hde->hne', omega, wk_h)

    in_maps = []
    for core in range(8):
        b, g = divmod(core, 4)
        sl = slice(512 * g, 512 * (g + 1))
        hsl = slice(4 * g, 4 * (g + 1))
        xq, xk, xv = xs[b]
        in_maps.append({
            "xq": xq, "xk": xk, "xv": xv,
            "wqom": _pad_feat(wqom[hsl]).astype(bf),
            "wkom": _pad_feat(wkom[hsl]).astype(bf),
            "wv": np.ascontiguousarray(w_v[sl, :].T).astype(bf),
            "wo": np.ascontiguousarray(w_o[:, sl].T).astype(bf),
            "mask": mask,
        })
    return in_maps


def kernel(q, k, v, w_q, w_k, w_v, w_o, omega):
    global LAST_EXEC_TIME_NS
    q, k, v = np.asarray(q), np.asarray(k), np.asarray(v)
    w_q, w_k, w_v, w_o = (np.asarray(a) for a in (w_q, w_k, w_v, w_o))
    omega = np.asarray(omega)

    if "nc" not in _CACHE:
        _CACHE["nc"] = _build()
    nc = _CACHE["nc"]

    in_maps = _prep_inputs(q, k, v, w_q, w_k, w_v, w_o, omega)
    trace = bool(os.environ.get("BASS_KERNEL_TRACE"))
    res = run_bass_kernel_spmd(nc, in_maps, core_ids=list(range(8)), trace=trace)
    LAST_EXEC_TIME_NS = res.exec_time_ns

    out = np.zeros((B, S, D), np.float32)
    for core in range(8):
        b = core // 4
        out[b] += res.results[core]["part"]
    return out


# revision 8
# speedup vs baseline: 1.3001x; 1.2254x over previous
"""Causal Performer (FAVOR+) Trainium2 kernel.

Sharding: 8 cores = 2 (batch) x 4 (head groups of 4 heads).  Each core
computes its 4 heads for one batch and returns a partial [4096, 2048]
output (its heads' contribution through w_o); the host sums the 4
partials per batch.

Key algebraic moves:
  - q/k head projections are FUSED with the random-feature map on the
    host: qf = (q @ Wq_h.T) @ omega.T == q @ (omega @ Wq_h).T, so the
    on-chip contraction produces only 8 features per head (32 per core)
    instead of 128 head dims -- 16x less PE work than materializing qh.
  - The reference's q-feature normalization cancels in num/den and is
    skipped; k-feature normalization is folded into the masked-A^T copy
    as a per-partition scale (scalar_tensor_tensor).
  - Causal scan is chunked: within a 512-block, masked A^T matmuls
    handle intra-block pairs; an [8, 129] running state (Z | z) carries
    history across blocks (updated via matmul, accumulated in fp32).

All matmuls bf16 with fp32 PSUM accumulation.
"""

import os
import numpy as np
import ml_dtypes

from concourse import bacc, mybir
import concourse.tile as tile
from concourse.bass import ts
from concourse.bass_utils import run_bass_kernel_spmd
from concourse.masks import make_identity

B, S, D = 2, 4096, 2048
H_PER = 4            # heads per core
DK = 128
NB = 8
NF = NB * H_PER      # 32 fused feature dims per core
SBLK = 512           # sequence block
NBLK = S // SBLK     # 8
NSUB = SBLK // 128   # 4 sub-chunks of 128
EPS = 1e-6

bf16 = mybir.dt.bfloat16
f32 = mybir.dt.float32

LAST_EXEC_TIME_NS = None
_CACHE = {}


def _build():
    nc = bacc.Bacc("TRN2", target_bir_lowering=False, debug=False)

    xq_d = nc.dram_tensor("xq", [D, S], bf16, kind="ExternalInput").ap()
    xk_d = nc.dram_tensor("xk", [D, S], bf16, kind="ExternalInput").ap()
    xv_d = nc.dram_tensor("xv", [D, S], bf16, kind="ExternalInput").ap()
    wqom_d = nc.dram_tensor("wqom", [D, 128], bf16, kind="ExternalInput").ap()
    wkom_d = nc.dram_tensor("wkom", [D, 128], bf16, kind="ExternalInput").ap()
    wv_d = nc.dram_tensor("wv", [D, 512], bf16, kind="ExternalInput").ap()
    wo_d = nc.dram_tensor("wo", [512, D], bf16, kind="ExternalInput").ap()
    mask_d = nc.dram_tensor("mask", [128, SBLK], f32, kind="ExternalInput").ap()
    part_d = nc.dram_tensor("part", [S, D], f32, kind="ExternalOutput").ap()

    KC = D // 128    # 16 contraction chunks

    with tile.TileContext(nc) as tc:
        with tc.tile_pool(name="const", bufs=1) as const, \
             tc.tile_pool(name="wpool", bufs=1) as wpool, \
             tc.tile_pool(name="state", bufs=1) as state, \
             tc.tile_pool(name="xpool", bufs=1) as xpool, \
             tc.tile_pool(name="vpool", bufs=2) as vpool, \
             tc.tile_pool(name="featpool", bufs=6) as featpool, \
             tc.tile_pool(name="atmpool", bufs=6) as atmpool, \
             tc.tile_pool(name="otpool", bufs=6) as otpool, \
             tc.tile_pool(name="osbpool", bufs=2) as osbpool, \
             tc.tile_pool(name="miscpool", bufs=6) as miscpool, \
             tc.tile_pool(name="psbig", bufs=5, space="PSUM") as psbig, \
             tc.tile_pool(name="pssml", bufs=3, space="PSUM") as pssml:

            ident = const.tile([128, 128], bf16, name="ident")
            make_identity(nc, ident)
            mask_sb = const.tile([128, SBLK], f32, name="mask_sb")
            nc.sync.dma_start(mask_sb[:], mask_d[:])
            ones_col = const.tile([128, 1], bf16, name="ones_col")
            nc.vector.memset(ones_col[:], 1.0)
            ones_row = const.tile([97, 128], bf16, name="ones_row")
            nc.vector.memset(ones_row[:], 1.0)

            wqom_sb = wpool.tile([128, KC, 128], bf16, name="wqom_sb")
            nc.sync.dma_start(wqom_sb[:], wqom_d.rearrange("(c p) m -> p c m", p=128))
            wkom_sb = wpool.tile([128, KC, 128], bf16, name="wkom_sb")
            nc.sync.dma_start(wkom_sb[:], wkom_d.rearrange("(c p) m -> p c m", p=128))
            wv_sb = wpool.tile([128, KC, 512], bf16, name="wv_sb")
            nc.sync.dma_start(wv_sb[:], wv_d.rearrange("(c p) m -> p c m", p=128))
            wo_sb = wpool.tile([128, H_PER, D], bf16, name="wo_sb")
            nc.sync.dma_start(wo_sb[:], wo_d.rearrange("(c p) m -> p c m", p=128))

            # persistent per-head scan state: cols 0:128 = Z, col 128 = z
            Zsb = []
            Zb16 = []
            for h in range(H_PER):
                zt = state.tile([NB, 132], f32, name=f"Zsb{h}")
                nc.vector.memset(zt[:], 0.0)
                Zsb.append(zt)
                zb = state.tile([NB, 132], bf16, name=f"Zb16_{h}")
                nc.vector.memset(zb[:], 0.0)
                Zb16.append(zb)

            for blk in range(NBLK):
                s0 = blk * SBLK

                xq_sb = xpool.tile([128, KC, SBLK], bf16, name=f"xq{blk}", tag="xq")
                nc.sync.dma_start(
                    xq_sb[:],
                    xq_d.rearrange("(c p) s -> p c s", p=128)[:, :, s0:s0 + SBLK])
                xk_sb = xpool.tile([128, KC, SBLK], bf16, name=f"xk{blk}", tag="xk")
                nc.sync.dma_start(
                    xk_sb[:],
                    xk_d.rearrange("(c p) s -> p c s", p=128)[:, :, s0:s0 + SBLK])
                xv_sb = xpool.tile([128, KC, SBLK], bf16, name=f"xv{blk}", tag="xv")
                nc.sync.dma_start(
                    xv_sb[:],
                    xv_d.rearrange("(c p) s -> p c s", p=128)[:, :, s0:s0 + SBLK])

                # ---- v projection: vha [s_sub(128), j, head, 132] (+ones col) ----
                vha = vpool.tile([128, NSUB, H_PER, 132], bf16, name=f"vha{blk}", tag="vha")
                for j in range(NSUB):
                    pp = psbig.tile([128, SBLK], f32, name=f"pv{blk}_{j}", tag="big")
                    for kc in range(KC):
                        nc.tensor.matmul(pp[:], xv_sb[:, kc, ts(j, 128)],
                                         wv_sb[:, kc, :],
                                         start=(kc == 0), stop=(kc == KC - 1))
                    nc.scalar.copy(vha[:, j, :, 0:128],
                                   pp.rearrange("p (h d) -> p h d", d=128))
                    nc.vector.memset(vha[:, j, :, 128:129], 1.0)

                # ---- fused q/k feature projections: [32, 512] ----
                qf_p = pssml.tile([128, SBLK], f32, name=f"qfp{blk}", tag="sml")
                kf_p = pssml.tile([128, SBLK], f32, name=f"kfp{blk}", tag="sml")
                for dst, wsb, xsb in ((qf_p, wqom_sb, xq_sb), (kf_p, wkom_sb, xk_sb)):
                    for kc in range(KC):
                        nc.tensor.matmul(dst[:], wsb[:, kc, :], xsb[:, kc, :],
                                         start=(kc == 0), stop=(kc == KC - 1))
                qsq = miscpool.tile([128, SBLK], f32, name=f"qsq{blk}", tag="qsq")
                nc.scalar.square(qsq[:], qf_p[:])
                ksq = miscpool.tile([128, SBLK], f32, name=f"ksq{blk}", tag="ksq")
                nc.scalar.square(ksq[:], kf_p[:])

                # per-head exp slices -> base-partition-0 tiles [8, 512]
                qfT = []
                kfTu = []
                for h in range(H_PER):
                    qt = featpool.tile([NB, SBLK], bf16, name=f"qfT{blk}_{h}", tag="qfT")
                    nc.scalar.activation(qt[:], qsq[32 * h:32 * h + NB, :],
                                         mybir.ActivationFunctionType.Exp, scale=-0.5)
                    qfT.append(qt)
                    kt = featpool.tile([NB, SBLK], bf16, name=f"kfT{blk}_{h}", tag="kfT")
                    nc.scalar.activation(kt[:], ksq[32 * h:32 * h + NB, :],
                                         mybir.ActivationFunctionType.Exp, scale=-0.5)
                    kfTu.append(kt)

                # ---- per-head scan ----
                # all heads' denominators accumulate into one bank (row 32h)
                den_all = pssml.tile([128, SBLK], f32, name=f"dena{blk}", tag="sml")
                outT = []
                numTs = []
                for h in range(H_PER):
                    # kf [s, nb] via PE transpose of kfTu; then normalizer
                    kfp2 = pssml.tile([128, NSUB, NB], bf16, name=f"kfp2{blk}_{h}", tag="sml")
                    for j in range(NSUB):
                        nc.tensor.transpose(kfp2[:, j, :], kfTu[h][:, ts(j, 128)],
                                            ident[0:NB, 0:NB])
                    kfu = featpool.tile([128, NSUB, NB], bf16, name=f"kfu{blk}_{h}", tag="kfu")
                    nc.vector.tensor_copy(kfu[:], kfp2[:])
                    ksum = miscpool.tile([128, NSUB], f32, name=f"ksum{blk}_{h}", tag="ksum")
                    nc.vector.reduce_sum(ksum[:], kfu[:], axis=mybir.AxisListType.X)
                    nc.vector.tensor_scalar_add(ksum[:], ksum[:], EPS)
                    krec = miscpool.tile([128, NSUB], f32, name=f"krec{blk}_{h}", tag="krec")
                    nc.vector.reciprocal(krec[:], ksum[:])
                    kfn = featpool.tile([128, NSUB, NB], bf16, name=f"kfn{blk}_{h}", tag="kfn")
                    for j in range(NSUB):
                        nc.vector.tensor_scalar(
                            out=kfn[:, j, :], in0=kfu[:, j, :],
                            scalar1=krec[:, j:j + 1], scalar2=None,
                            op0=mybir.AluOpType.mult)

                    # masked A^T blocks, normalization folded in as row scale
                    atm = []
                    for i2 in range(NSUB):
                        n_i = SBLK - 128 * i2
                        at_p = psbig.tile([128, n_i], f32, name=f"at{blk}_{h}_{i2}", tag="big")
                        nc.tensor.matmul(at_p[:], kfTu[h][:, ts(i2, 128)],
                                         qfT[h][:, 128 * i2:SBLK], start=True, stop=True)
                        am = atmpool.tile([128, SBLK], bf16, name=f"am{blk}_{h}_{i2}", tag="atm")
                        nc.vector.scalar_tensor_tensor(
                            out=am[:, :n_i], in0=at_p[:], scalar=krec[:, i2:i2 + 1],
                            in1=mask_sb[:, :n_i],
                            op0=mybir.AluOpType.mult, op1=mybir.AluOpType.mult)
                        atm.append(am)

                    # numerator^T [d, s]
                    numT = psbig.tile([128, SBLK], f32, name=f"numT{blk}_{h}", tag="big")
                    for i2 in range(NSUB):
                        nc.tensor.matmul(numT[:, 128 * i2:SBLK],
                                         vha[:, i2, h, 0:128], atm[i2][:, :SBLK - 128 * i2],
                                         start=(i2 == 0), stop=False)
                    nc.tensor.matmul(numT[:], Zb16[h][:, 0:128], qfT[h][:],
                                     start=False, stop=True)

                    # denominator -> den_all row 32h
                    dr = den_all[32 * h:32 * h + 1, :]
                    for i2 in range(NSUB):
                        nc.tensor.matmul(dr[:, 128 * i2:SBLK],
                                         ones_col[:], atm[i2][:, :SBLK - 128 * i2],
                                         start=(i2 == 0), stop=False,
                                         tile_position=(0, 32 * h))
                    nc.tensor.matmul(dr[:], Zb16[h][:, 128:129], qfT[h][:],
                                     start=False, stop=True,
                                     tile_position=(0, 32 * h))
                    numc = miscpool.tile([128, SBLK], bf16, name=f"numc{blk}_{h}", tag="numc")
                    nc.vector.tensor_copy(numc[:], numT[:])
                    numTs.append(numc)

                    # state update (reads of Zb16 above precede these writes)
                    su = pssml.tile([NB, 132], f32, name=f"su{blk}_{h}", tag="sml")
                    for i2 in range(NSUB):
                        nc.tensor.matmul(su[:, 0:129], kfn[:, i2, :],
                                         vha[:, i2, h, 0:129],
                                         start=(i2 == 0), stop=(i2 == NSUB - 1))
                    nc.vector.tensor_add(Zsb[h][:, 0:129], Zsb[h][:, 0:129], su[:, 0:129])
                    nc.vector.tensor_copy(Zb16[h][:, 0:129], Zsb[h][:, 0:129])

                # one recip chain per block over all 4 heads (rows 0,32,64,96)
                drr = miscpool.tile([97, SBLK], f32, name=f"drr{blk}", tag="drr")
                nc.vector.tensor_scalar_add(drr[:], den_all[0:97, :], EPS)
                nc.vector.reciprocal(drr[:], drr[:])
                drb = miscpool.tile([97, SBLK], bf16, name=f"drb{blk}", tag="drb")
                nc.vector.tensor_copy(drb[:], drr[:])

                for h in range(H_PER):
                    # broadcast recip across partitions and divide
                    bc_p = psbig.tile([128, SBLK], f32, name=f"bcp{blk}_{h}", tag="big")
                    nc.tensor.matmul(bc_p[:], ones_row[32 * h:32 * h + 1, :],
                                     drb[32 * h:32 * h + 1, :],
                                     start=True, stop=True,
                                     tile_position=(32 * h, 0))
                    oT = otpool.tile([128, SBLK], bf16, name=f"oT{blk}_{h}", tag="outT")
                    nc.vector.tensor_mul(oT[:], bc_p[:], numTs[h][:])
                    outT.append(oT)

                # ---- output projection ----
                for j in range(NSUB):
                    osb = osbpool.tile([128, D], f32, name=f"osb{blk}_{j}", tag="osb")
                    for c in range(4):
                        op = psbig.tile([128, 512], f32, name=f"op{blk}_{j}_{c}", tag="big")
                        for h in range(H_PER):
                            nc.tensor.matmul(op[:], outT[h][:, ts(j, 128)],
                                             wo_sb[:, h, ts(c, 512)],
                                             start=(h == 0), stop=(h == H_PER - 1))
                        nc.scalar.copy(osb[:, ts(c, 512)], op[:])
                    r0 = s0 + 128 * j
                    nc.sync.dma_start(part_d[r0:r0 + 128, :], osb[:])

    nc.compile()
    return nc


def _pad_feat(w):
    """[4, 8, D] head-feature weights -> [D, 128] with head h at cols 32h."""
    out = np.zeros((128, D), np.float32)
    for h in range(H_PER):
        out[32 * h:32 * h + NB] = w[h]
    return np.ascontiguousarray(out.T)


def _prep_inputs(q, k, v, w_q, w_k, w_v, w_o, omega):
    """Host-side sharding: returns in_maps for the 8 cores."""
    bf = ml_dtypes.bfloat16
    mask = np.ones((128, SBLK), np.float32)
    mask[:, :128] = np.triu(np.ones((128, 128), np.float32))

    xs = []
    for b in range(B):
        xs.append((np.ascontiguousarray(q[b].T).astype(bf),
                   np.ascontiguousarray(k[b].T).astype(bf),
                   np.ascontiguousarray(v[b].T).astype(bf)))

    # fused feature projections: per head, omega @ Wq_head  -> [8, 2048]
    # stacked per group -> [32, 2048], transposed -> [2048, 32]
    wq_h = w_q.reshape(16, DK, D)                 # [head, dk, d_in]
    wk_h = w_k.reshape(16, DK, D)
    wqom = np.einsum('nd,hde->hne', omega, wq_h)  # [16, 8, D]
    wkom = np.einsum('nd,hde->hne', omega, wk_h)

    in_maps = []
    for core in range(8):
        b, g = divmod(core, 4)
        sl = slice(512 * g, 512 * (g + 1))
        hsl = slice(4 * g, 4 * (g + 1))
        xq, xk, xv = xs[b]
        in_maps.append({
            "xq": xq, "xk": xk, "xv": xv,
            "wqom": _pad_feat(wqom[hsl]).astype(bf),
            "wkom": _pad_feat(wkom[hsl]).astype(bf),
            "wv": np.ascontiguousarray(w_v[sl, :].T).astype(bf),
            "wo": np.ascontiguousarray(w_o[:, sl].T).astype(bf),
            "mask": mask,
        })
    return in_maps


def kernel(q, k, v, w_q, w_k, w_v, w_o, omega):
    global LAST_EXEC_TIME_NS
    q, k, v = np.asarray(q), np.asarray(k), np.asarray(v)
    w_q, w_k, w_v, w_o = (np.asarray(a) for a in (w_q, w_k, w_v, w_o))
    omega = np.asarray(omega)

    if "nc" not in _CACHE:
        _CACHE["nc"] = _build()
    nc = _CACHE["nc"]

    in_maps = _prep_inputs(q, k, v, w_q, w_k, w_v, w_o, omega)
    trace = bool(os.environ.get("BASS_KERNEL_TRACE"))
    res = run_bass_kernel_spmd(nc, in_maps, core_ids=list(range(8)), trace=trace)
    LAST_EXEC_TIME_NS = res.exec_time_ns

    out = np.zeros((B, S, D), np.float32)
    for core in range(8):
        b = core // 4
        out[b] += res.results[core]["part"]
    return out
